# revision 7
# baseline (speedup 1.0000x reference)
# Trainium2 Bass kernel for nn_DriftDiT (DiT block: adaLN + self-attn w/ QK-norm
# + RoPE + cross-attn + SwiGLU MLP), distributed over 8 NeuronCores.
#
# Sharding: sequence-parallel over tokens (8 x 256). All weights are replicated
# in bf16 (memory regime: weight streaming dominates). Self-attention K/V are
# computed per-core for the local tokens and exchanged with two AllGathers
# (heads 0-5, then heads 6-11) so score/AV matmuls overlap the second gather.
#
# On-chip layout is feature-major ("x^T": [D, tokens] with partition = feature).
# Attention uses transposed scores s^T[j, i] = k_j . q_i so the exp'd scores can
# feed the AV matmul directly as the moving operand (no transposes), with a
# ones-column appended to V to produce softmax denominators for free. RMS-norm
# partition reductions run on the PE via (block-)ones matmuls, which also
# broadcasts the result to all partitions. exp() needs no max-subtraction:
# |q.k|/8 <= 8 after QK RMS-norm (Cauchy-Schwarz).
import sys

sys.path.insert(0, "/opt/trn_rl_repo")

from contextlib import ExitStack

import numpy as np
import ml_dtypes

import concourse.bass as bass  # noqa: F401
import concourse.bacc as bacc
import concourse.mybir as mybir
import concourse.tile as tile
from concourse.bass_utils import run_bass_kernel_spmd

BF16_NP = ml_dtypes.bfloat16
BF = mybir.dt.bfloat16
F32 = mybir.dt.float32
AF = mybir.ActivationFunctionType

NCORES = 8
D = 768
KT = 6            # 128-row tiles over D
N = 2048
NT = N // NCORES  # tokens per core
H = 12
HD = 64
MH = 3072
NCTOK = 16
EPS = 1e-6
SCALE = 0.125     # hd^-0.5
KVA = 3 * 128 * NT   # k half of one AG payload (elements)
VPF = 6 * 65         # v columns per AG group (6 heads x 65)
KVB = 2 * 128 * VPF  # v half
KVTOT = KVA + KVB


def _declare_io(nc):
    t = {}

    def di(name, shape, dt=F32):
        t[name] = nc.dram_tensor(name, list(shape), dt, kind="ExternalInput")

    di("xT", (D, NT))
    di("cvec", (128, KT))
    di("condT", (D, NCTOK))
    di("adaT", (9, D, D), BF)
    di("adab", (128, 9 * KT))
    di("qkvT", (D, 3 * D), BF)
    di("projT", (D, D), BF)
    di("caqT", (D, D), BF)
    di("cakT", (D, D), BF)
    di("cavT", (D, D), BF)
    di("caprojT", (D, D), BF)
    di("w1T", (D, MH), BF)
    di("w3T", (D, MH), BF)
    di("w2T", (MH, D), BF)
    for nm in ("n1w", "n2w", "n3w", "cnw", "psab", "pcab"):
        di(nm, (128, KT))
    for nm in ("cosq", "sinq", "cosk", "sink", "cosqc", "sinqc"):
        di(nm, (128, NT))
    di("caknw", (128, 1))
    di("bones64", (128, 128), BF)
    di("bones2", (2, 128), BF)
    di("ones128", (128, 128), BF)
    di("ident", (128, 128), BF)
    t["outT"] = nc.dram_tensor("outT", [D, NT], F32, kind="ExternalOutput")
    return t


def _r6(ap):
    # [k*128, F] dram -> [128, k, F] (partition, ktile, free)
    return ap.rearrange("(k p) f -> p k f", p=128)


def build_program():
    nc = bacc.Bacc(num_devices=NCORES)
    io = _declare_io(nc)

    with tile.TileContext(nc) as tc, ExitStack() as ctx:
        pers = ctx.enter_context(tc.tile_pool(name="pers", bufs=1))
        scr = ctx.enter_context(tc.tile_pool(name="scr", bufs=2))
        dram = ctx.enter_context(tc.tile_pool(name="dram", bufs=1, space="DRAM"))

        # ---------- persistent tiles ----------
        x = pers.tile([128, KT, NT], F32)
        nc.sync.dma_start(x[:], _r6(io["xT"][:]))
        mod = pers.tile([128, 9 * KT], F32)
        oT = pers.tile([128, KT, NT], BF)

        small = {}
        for nm in ("cvec", "adab", "n1w", "n2w", "n3w", "cnw", "psab", "pcab",
                   "caknw", "bones64", "bones2", "ones128", "ident"):
            st = pers.tile(list(io[nm].shape), io[nm].dtype, name=f"sb_{nm}")
            nc.sync.dma_start(st[:], io[nm][:])
            small[nm] = st
        bones64, bones2, ones128, ident = (
            small["bones64"], small["bones2"], small["ones128"], small["ident"])
        adab = small["adab"]

        silu_c = pers.tile([128, KT], BF)
        sg_c = pers.tile([128, KT], F32)
        nc.scalar.activation(sg_c[:], small["cvec"][:], AF.Sigmoid)
        nc.vector.tensor_mul(silu_c[:], sg_c[:], small["cvec"][:])
        epsb = pers.tile([128, 1], F32)
        nc.gpsimd.memset(epsb[:], EPS)

        # ---------- helpers ----------
        def ada_group(g, wpool, pspool, ps_tag, ps_bufs, w_bufs=2):
            """mod[:, 6g:6g+6] = silu(c) @ ada_w.T chunk g (+ bias)."""
            wg = wpool.tile([128, KT, D], BF, tag="adaw", bufs=w_bufs,
                            name=f"adaw{g}")
            nc.sync.dma_start(wg[:], _r6(io["adaT"][g]))
            for ft in range(KT):
                ps = pspool.tile([128, NT], F32, tag=ps_tag, bufs=ps_bufs,
                                 name=f"adap{g}_{ft}")
                for k in range(KT):
                    nc.tensor.matmul(ps[0:128, 0:1],
                                     wg[:, k, ft * 128:(ft + 1) * 128],
                                     silu_c[:, k:k + 1],
                                     start=(k == 0), stop=(k == KT - 1))
                col = g * KT + ft
                nc.scalar.activation(mod[:, col:col + 1], ps[0:128, 0:1],
                                     AF.Identity, bias=adab[:, col:col + 1])

        def norm_mod(wv, sh_off, sc_off, h, pspool):
            """h (bf16 [128,6,NT]) = modulate(rms(x, wv), mod sh/sc chunks)."""
            ms = pspool.tile([128, NT], F32, tag="red", bufs=2,
                             name=f"ms{sh_off}")
            for k in range(KT):
                sq = scr.tile([128, NT], BF, tag="sq", name=f"sq{sh_off}_{k}")
                nc.scalar.activation(sq[:], x[:, k], AF.Square)
                nc.tensor.matmul(ms[:], ones128[:], sq[:],
                                 start=(k == 0), stop=(k == KT - 1))
            rs = scr.tile([128, NT], F32, tag="rs", name=f"rs{sh_off}")
            nc.scalar.activation(rs[:], ms[:], AF.Sqrt, bias=epsb[:], scale=1.0 / D)
            inv = scr.tile([128, NT], F32, tag="inv", name=f"inv{sh_off}")
            nc.vector.reciprocal(inv[:], rs[:])
            a = scr.tile([128, KT], F32, tag="amod", name=f"amod{sh_off}")
            nc.vector.tensor_scalar_add(a[:], mod[:, sc_off:sc_off + KT], 1.0)
            nc.vector.tensor_mul(a[:], a[:], wv[:])
            for k in range(KT):
                t = scr.tile([128, NT], F32, tag="tnorm", name=f"tn{sh_off}_{k}")
                nc.vector.tensor_mul(t[:], x[:, k], inv[:])
                nc.scalar.activation(h[:, k], t[:], AF.Identity,
                                     bias=mod[:, sh_off + k:sh_off + k + 1],
                                     scale=a[:, k:k + 1])

        def qknorm_rope(ps, co, si, out_bf, pspool, uid):
            """per-head RMS over 64 rows + RoPE via folded tables; psum -> bf16."""
            sq2 = scr.tile([128, NT], BF, tag="sq2", name=f"sq2_{uid}")
            nc.scalar.activation(sq2[:], ps[:], AF.Square)
            msh = pspool.tile([128, NT], F32, tag="red", bufs=2,
                              name=f"msh_{uid}")
            nc.tensor.matmul(msh[:], bones64[:], sq2[:], start=True, stop=True)
            rsh = scr.tile([128, NT], F32, tag="rsh", name=f"rsh_{uid}")
            nc.scalar.activation(rsh[:], msh[:], AF.Sqrt, bias=epsb[:], scale=1.0 / HD)
            ivh = scr.tile([128, NT], F32, tag="ivh", name=f"ivh_{uid}")
            nc.vector.reciprocal(ivh[:], rsh[:])
            t = scr.tile([128, NT], F32, tag="tq", name=f"tq_{uid}")
            nc.vector.tensor_mul(t[:], ps[:], ivh[:])
            c1 = scr.tile([128, NT], F32, tag="c1", name=f"c1_{uid}")
            nc.vector.tensor_mul(c1[:], t[:], co[:])
            sw = scr.tile([128, NT], F32, tag="sw", name=f"sw_{uid}")
            for (a0, b0) in ((0, 32), (32, 0), (64, 96), (96, 64)):
                nc.gpsimd.tensor_copy(sw[a0:a0 + 32, :], t[b0:b0 + 32, :])
            m2 = scr.tile([128, NT], F32, tag="m2", name=f"m2_{uid}")
            nc.vector.tensor_mul(m2[:], sw[:], si[:])
            nc.vector.tensor_add(out_bf, c1[:], m2[:])

        def qk_rms_only(ps, wvec, out_bf, pspool, uid):
            """per-head RMS over 64 rows (no rope), scaled by wvec [128,1]."""
            nfree = ps.shape[-1]
            sq2 = scr.tile([128, nfree], BF, tag="sq2c", name=f"sq2c_{uid}")
            nc.scalar.activation(sq2[:], ps[:], AF.Square)
            msh = pspool.tile([128, NT], F32, tag="red", bufs=2,
                              name=f"mshc_{uid}")
            nc.tensor.matmul(msh[0:128, 0:nfree], bones64[:], sq2[:],
                             start=True, stop=True)
            rsh = scr.tile([128, nfree], F32, tag="rshc", name=f"rshc_{uid}")
            nc.scalar.activation(rsh[:], msh[0:128, 0:nfree], AF.Sqrt,
                                 bias=epsb[:], scale=1.0 / HD)
            ivh = scr.tile([128, nfree], F32, tag="ivhc", name=f"ivhc_{uid}")
            nc.vector.reciprocal(ivh[:], rsh[:])
            t = scr.tile([128, nfree], F32, tag="tqc", name=f"tqc_{uid}")
            nc.vector.tensor_mul(t[:], ps[:], ivh[:])
            nc.scalar.activation(out_bf, t[:], AF.Copy, scale=wvec)

        def linear6(wsb, rhs6, ft, pspool, uid, lin_bufs, nacc=KT, nfree=NT):
            """psum [128,nfree] = sum_k wsb[:,k,ft*128:+128].T @ rhs6[:,k]."""
            ps = pspool.tile([128, NT], F32, tag="lin", bufs=lin_bufs,
                             name=f"lin_{uid}")
            for k in range(nacc):
                nc.tensor.matmul(ps[0:128, 0:nfree],
                                 wsb[:, k, ft * 128:(ft + 1) * 128],
                                 rhs6[:, k],
                                 start=(k == 0), stop=(k == nacc - 1))
            return ps[0:128, 0:nfree]

        def proj_residual(wname, rhs6, g_off, bias_vec, pspool, wpool,
                          lin_bufs, uid):
            """x += g * (W @ rhs + b); bias_vec may be None (mlp)."""
            pw = wpool.tile([128, KT, D], BF, tag="projw", bufs=2,
                            name=f"pw_{uid}")
            nc.sync.dma_start(pw[:], _r6(io[wname][:]))
            gb = None
            if bias_vec is not None:
                gb = scr.tile([128, KT], F32, tag="gb", name=f"gb_{uid}")
                nc.vector.tensor_mul(gb[:], mod[:, g_off:g_off + KT], bias_vec[:])
            for ft in range(KT):
                pp = linear6(pw, rhs6, ft, pspool, f"{uid}_{ft}", lin_bufs)
                t = scr.tile([128, NT], F32, tag="tres", name=f"tres_{uid}_{ft}")
                if bias_vec is not None:
                    nc.scalar.activation(t[:], pp, AF.Identity,
                                         bias=gb[:, ft:ft + 1],
                                         scale=mod[:, g_off + ft:g_off + ft + 1])
                else:
                    nc.scalar.activation(t[:], pp, AF.Copy,
                                         scale=mod[:, g_off + ft:g_off + ft + 1])
                nc.vector.tensor_add(x[:, ft], x[:, ft], t[:])

        def attention_pair(K, V, qsrc, mloc, mglob, out_tile, pspool, njb,
                           s_bufs, o_bufs, uid):
            """heads (2*mglob, 2*mglob+1): transposed-score flash attention."""
            o_list = []
            for h2 in range(2):
                o_ps = pspool.tile([65, NT], F32, tag="attn_o", bufs=o_bufs,
                                   name=f"ops_{uid}_{h2}")
                lh = 2 * mloc + h2
                for jb in range(njb):
                    s_ps = pspool.tile([128, NT], F32, tag="attn_s",
                                       bufs=s_bufs, name=f"sps_{uid}_{h2}_{jb}")
                    if njb > 1:
                        klhs = K[h2 * 64:(h2 + 1) * 64, mloc,
                                 jb * 128:(jb + 1) * 128]
                        vsl = V[:, jb, lh * 65:(lh + 1) * 65]
                        nj = 128
                    else:
                        klhs = K[h2 * 64:(h2 + 1) * 64, mloc, :]
                        vsl = V[:, lh * 65:(lh + 1) * 65]
                        nj = NCTOK
                    nc.tensor.matmul(s_ps[0:nj, :], klhs,
                                     qsrc[h2 * 64:(h2 + 1) * 64, mglob],
                                     start=True, stop=True)
                    p_bf = scr.tile([128, NT], BF, tag="pexp",
                                    name=f"p_{uid}_{h2}_{jb}")
                    nc.scalar.activation(p_bf[0:nj, :], s_ps[0:nj, :], AF.Exp,
                                         scale=SCALE)
                    nc.tensor.matmul(o_ps[:], vsl, p_bf[0:nj, :],
                                     start=(jb == 0), stop=(jb == njb - 1),
                                     skip_group_check=True)
                o_list.append(o_ps)
            for h2 in range(2):
                o_ps = o_list[h2]
                rsum = scr.tile([1, NT], F32, tag="rsum", name=f"rsum_{uid}_{h2}")
                nc.vector.reciprocal(rsum[:], o_ps[64:65, :])
                rsbf = scr.tile([1, NT], BF, tag="rsbf", name=f"rsbf_{uid}_{h2}")
                nc.scalar.activation(rsbf[:], rsum[:], AF.Copy)
                rb_ps = pspool.tile([128, NT], F32, tag="attn_s", bufs=s_bufs,
                                    name=f"rb_{uid}_{h2}")
                nc.tensor.matmul(rb_ps[0:64, :], ones128[0:1, 0:64], rsbf[:],
                                 start=True, stop=True)
                rb = scr.tile([64, NT], F32, tag="rb", name=f"rbs_{uid}_{h2}")
                nc.scalar.activation(rb[:], rb_ps[0:64, :], AF.Copy)
                nc.vector.tensor_mul(out_tile[h2 * 64:(h2 + 1) * 64, mglob],
                                     o_ps[0:64, :], rb[:])

        # =================== stage 1: adaLN + qkv + KV allgather ===============
        kva_in = dram.tile([KVTOT], BF)
        kvb_in = dram.tile([KVTOT], BF)
        kva_out = dram.tile([NCORES * KVTOT], BF, addr_space="Shared")
        kvb_out = dram.tile([NCORES * KVTOT], BF, addr_space="Shared")

        with tc.tile_pool(name="sap", bufs=1) as sap:
            h1 = sap.tile([128, KT, NT], BF)
            qq = sap.tile([128, KT, NT], BF)
            kbuild = sap.tile([128, KT, NT], BF)
            vtmp = sap.tile([128, KT, NT], BF)
            vbuild = sap.tile([128, 2, 12 * 65], BF)
            nc.vector.memset(
                vbuild[:].rearrange("p j (h c) -> p j h c", c=65)[:, :, :, 64:65],
                1.0)
            rope = {}
            for nm in ("cosq", "sinq", "cosk", "sink"):
                rt = sap.tile([128, NT], F32, name=f"sb_{nm}")
                nc.sync.dma_start(rt[:], io[nm][:])
                rope[nm] = rt

            # --------------- stage 1 ---------------
            with tc.tile_pool(name="ps1", space="PSUM", bufs=1) as ps1, \
                    tc.tile_pool(name="wq", bufs=1) as wqpool:
                ada_group(0, wqpool, ps1, "ada", 1)   # sh_msa
                ada_group(1, wqpool, ps1, "ada", 1)   # sc_msa
                norm_mod(small["n1w"], 0, KT, h1, ps1)

                qkvw = wqpool.tile([128, KT, 3 * D], BF)
                nc.sync.dma_start(qkvw[:], _r6(io["qkvT"][:]))

                def qkv_ft(ft):
                    ps = linear6(qkvw, h1, ft, ps1, f"qkv{ft}", 3)
                    if 6 <= ft < 12:
                        qknorm_rope(ps, rope["cosk"], rope["sink"],
                                    kbuild[:, ft - 6], ps1, f"k{ft}")
                    elif ft >= 12:
                        m = ft - 12
                        nc.scalar.activation(vtmp[:, m], ps, AF.Copy)
                        for jt in range(2):
                            tp = ps1.tile([128, 128], BF, tag="tr", bufs=2,
                                          name=f"vtr{m}_{jt}")
                            nc.tensor.transpose(
                                tp[:], vtmp[:, m, jt * 128:(jt + 1) * 128],
                                ident[:])
                            dst = vbuild[:, jt].rearrange(
                                "p (h c) -> p h c", c=65)
                            nc.scalar.activation(
                                dst[:, 2 * m:2 * m + 2, 0:64],
                                tp[:].rearrange("p (h c) -> p h c", c=64),
                                AF.Copy)
                    else:
                        qknorm_rope(ps, rope["cosq"], rope["sinq"],
                                    qq[:, ft], ps1, f"q{ft}")

                def kv_dma(group, kv_in):
                    lo = 3 * group
                    nc.sync.dma_start(
                        kv_in[0:KVA].rearrange("(k p t) -> p k t", p=128, t=NT),
                        kbuild[:, lo:lo + 3])
                    nc.sync.dma_start(
                        kv_in[KVA:KVTOT].rearrange("(j p f) -> p j f",
                                                   p=128, f=VPF),
                        vbuild[:, :, group * VPF:(group + 1) * VPF])

                for ft in (6, 7, 8, 12, 13, 14):
                    qkv_ft(ft)
                kv_dma(0, kva_in)
                nc.gpsimd.collective_compute(
                    "AllGather", mybir.AluOpType.bypass,
                    replica_groups=[list(range(NCORES))],
                    ins=[kva_in[:].opt()], outs=[kva_out[:].opt()])
                for ft in (9, 10, 11, 15, 16, 17):
                    qkv_ft(ft)
                kv_dma(1, kvb_in)
                nc.gpsimd.collective_compute(
                    "AllGather", mybir.AluOpType.bypass,
                    replica_groups=[list(range(NCORES))],
                    ins=[kvb_in[:].opt()], outs=[kvb_out[:].opt()])
                for ft in range(6):
                    qkv_ft(ft)
                ada_group(2, wqpool, ps1, "ada", 1)   # g_msa (filler during AG)

            # --------------- stage 2: self-attention + proj ---------------
            with tc.tile_pool(name="ps2", space="PSUM", bufs=1) as ps2, \
                    tc.tile_pool(name="kv", bufs=1) as kvpool, \
                    tc.tile_pool(name="w2p", bufs=1) as w2pool:
                Kab, Vab = [], []
                for g, kv_out in ((0, kva_out), (1, kvb_out)):
                    Ksb = kvpool.tile([128, 3, N], BF, name=f"Ksb{g}")
                    Vsb = kvpool.tile([128, 16, VPF], BF, name=f"Vsb{g}")
                    for r in range(NCORES):
                        base = r * KVTOT
                        nc.sync.dma_start(
                            Ksb[:, :, r * NT:(r + 1) * NT],
                            kv_out[base:base + KVA].rearrange(
                                "(k p t) -> p k t", p=128, t=NT))
                        nc.sync.dma_start(
                            Vsb[:, 2 * r:2 * r + 2, :],
                            kv_out[base + KVA:base + KVTOT].rearrange(
                                "(j p f) -> p j f", p=128, f=VPF))
                    Kab.append(Ksb)
                    Vab.append(Vsb)
                for m in range(6):
                    g = 0 if m < 3 else 1
                    attention_pair(Kab[g], Vab[g], qq, m % 3, m, oT, ps2,
                                   16, 3, 2, f"sa{m}")
                proj_residual("projT", oT, 2 * KT, small["psab"], ps2, w2pool,
                              3, "sa")

        # =================== stage 3: cross-attention ==========================
        with tc.tile_pool(name="ps3", space="PSUM", bufs=1) as ps3, \
                tc.tile_pool(name="w3p", bufs=1) as w3pool:
            ada_group(3, w3pool, ps3, "lin", 2)   # sh_ca
            ada_group(4, w3pool, ps3, "lin", 2)   # sc_ca
            h2b = w3pool.tile([128, KT, NT], BF, name="h2b")
            norm_mod(small["n2w"], 3 * KT, 4 * KT, h2b, ps3)

            qca = w3pool.tile([128, KT, NT], BF, name="qca")
            kca = w3pool.tile([128, KT, NCTOK], BF, name="kca")
            vcab = w3pool.tile([NCTOK, 12 * 65], BF, name="vcab")
            nc.vector.memset(
                vcab[:].rearrange("p (h c) -> p h c", c=65)[:, :, 64:65], 1.0)
            ropec = {}
            for nm in ("cosqc", "sinqc"):
                rt = w3pool.tile([128, NT], F32, name=f"sb_{nm}")
                nc.sync.dma_start(rt[:], io[nm][:])
                ropec[nm] = rt

            # cond norm (tiny, replicated on every core)
            csb = w3pool.tile([128, KT, NCTOK], F32, name="csb")
            nc.sync.dma_start(csb[:], _r6(io["condT"][:]))
            msc = ps3.tile([128, NT], F32, tag="red", bufs=2, name="msc")
            for k in range(KT):
                sqc = scr.tile([128, NCTOK], BF, tag="sqcn", name=f"sqcn{k}")
                nc.scalar.activation(sqc[:], csb[:, k], AF.Square)
                nc.tensor.matmul(msc[0:128, 0:NCTOK], ones128[:], sqc[:],
                                 start=(k == 0), stop=(k == KT - 1))
            rsc = scr.tile([128, NCTOK], F32, tag="rscn", name="rscn")
            nc.scalar.activation(rsc[:], msc[0:128, 0:NCTOK], AF.Sqrt,
                                 bias=epsb[:], scale=1.0 / D)
            ivc = scr.tile([128, NCTOK], F32, tag="ivcn", name="ivcn")
            nc.vector.reciprocal(ivc[:], rsc[:])
            cnb = w3pool.tile([128, KT, NCTOK], BF, name="cnb")
            for k in range(KT):
                tcn = scr.tile([128, NCTOK], F32, tag="tcn", name=f"tcn{k}")
                nc.vector.tensor_mul(tcn[:], csb[:, k], ivc[:])
                nc.scalar.activation(cnb[:, k], tcn[:], AF.Copy,
                                     scale=small["cnw"][:, k:k + 1])

            cakw = w3pool.tile([128, KT, D], BF, tag="caw", bufs=2, name="cakw")
            nc.sync.dma_start(cakw[:], _r6(io["cakT"][:]))
            cavw = w3pool.tile([128, KT, D], BF, tag="caw", bufs=2, name="cavw")
            nc.sync.dma_start(cavw[:], _r6(io["cavT"][:]))
            vcat = w3pool.tile([128, KT, NCTOK], BF, name="vcat")
            for ft in range(KT):
                psk = linear6(cakw, cnb, ft, ps3, f"cak{ft}", 2, nfree=NCTOK)
                qk_rms_only(psk, small["caknw"][:, 0:1], kca[:, ft], ps3,
                            f"cak{ft}")
                psv = linear6(cavw, cnb, ft, ps3, f"cav{ft}", 2, nfree=NCTOK)
                nc.scalar.activation(vcat[:, ft], psv, AF.Copy)
                tpv = ps3.tile([NCTOK, 128], BF, tag="tr", bufs=1,
                               name=f"vctr{ft}")
                nc.tensor.transpose(tpv[:], vcat[:, ft], ident[:])
                dst = vcab[:].rearrange("p (h c) -> p h c", c=65)
                nc.scalar.activation(dst[:, 2 * ft:2 * ft + 2, 0:64],
                                     tpv[:].rearrange("p (h c) -> p h c", c=64),
                                     AF.Copy)

            caqw = w3pool.tile([128, KT, D], BF, tag="caw", bufs=2, name="caqw")
            nc.sync.dma_start(caqw[:], _r6(io["caqT"][:]))
            for ft in range(KT):
                psq = linear6(caqw, h2b, ft, ps3, f"caq{ft}", 2)
                qknorm_rope(psq, ropec["cosqc"], ropec["sinqc"],
                            qca[:, ft], ps3, f"cq{ft}")

            ada_group(5, w3pool, ps3, "lin", 2)   # g_ca
            for m in range(6):
                attention_pair(kca, vcab, qca, m, m, oT, ps3, 1, 1, 2, f"ca{m}")
            proj_residual("caprojT", oT, 5 * KT, small["pcab"], ps3, w3pool,
                          2, "ca")

        # =================== stage 4: SwiGLU MLP ===============================
        with tc.tile_pool(name="ps4", space="PSUM", bufs=1) as ps4, \
                tc.tile_pool(name="w4p", bufs=1) as w4pool:
            ada_group(6, w4pool, ps4, "lin", 6, w_bufs=1)   # sh_mlp
            ada_group(7, w4pool, ps4, "lin", 6, w_bufs=1)   # sc_mlp
            h3b = w4pool.tile([128, KT, NT], BF, name="h3b")
            norm_mod(small["n3w"], 6 * KT, 7 * KT, h3b, ps4)

            w1w = w4pool.tile([128, KT, MH], BF, name="w1w")
            nc.sync.dma_start(w1w[:], _r6(io["w1T"][:]))
            w3w = w4pool.tile([128, KT, MH], BF, name="w3w")
            nc.sync.dma_start(w3w[:], _r6(io["w3T"][:]))
            h13 = w4pool.tile([128, 24, NT], BF, name="h13")
            for ft in range(24):
                p1 = linear6(w1w, h3b, ft, ps4, f"w1_{ft}", 6)
                p3 = linear6(w3w, h3b, ft, ps4, f"w3_{ft}", 6)
                sg = scr.tile([128, NT], BF, tag="sg", name=f"sg_{ft}")
                nc.scalar.activation(sg[:], p1, AF.Sigmoid)
                s1 = scr.tile([128, NT], BF, tag="silu", name=f"s1_{ft}")
                nc.vector.tensor_mul(s1[:], sg[:], p1)
                nc.vector.tensor_mul(h13[:, ft], s1[:], p3)

            ada_group(8, w4pool, ps4, "lin", 6, w_bufs=1)   # g_mlp
            w2w = w4pool.tile([128, 24, D], BF, name="w2w")
            nc.sync.dma_start(w2w[:], io["w2T"][:].rearrange(
                "(k p) f -> p k f", p=128))
            for ft in range(KT):
                pp = ps4.tile([128, NT], F32, tag="lin", bufs=6, name=f"mo{ft}")
                for k in range(24):
                    nc.tensor.matmul(pp[:], w2w[:, k, ft * 128:(ft + 1) * 128],
                                     h13[:, k], start=(k == 0), stop=(k == 23))
                t = scr.tile([128, NT], F32, tag="tres", name=f"tmo{ft}")
                nc.scalar.activation(t[:], pp[:], AF.Copy,
                                     scale=mod[:, 8 * KT + ft:8 * KT + ft + 1])
                nc.vector.tensor_add(x[:, ft], x[:, ft], t[:])
                nc.sync.dma_start(
                    io["outT"][ft * 128:(ft + 1) * 128, :], x[:, ft])

    nc.compile()
    return nc


# ============================ host-side prep ==================================

def prep_inputs(inputs):
    f32 = np.float32
    x = np.asarray(inputs["x"], f32)[0]
    c = np.asarray(inputs["c"], f32)[0]
    cond = np.asarray(inputs["cond"], f32)[0]
    cos_t = np.asarray(inputs["rope_cos"], f32)
    sin_t = np.asarray(inputs["rope_sin"], f32)

    def bfT(w):
        return np.ascontiguousarray(np.asarray(w, f32).T).astype(BF16_NP)

    def fm(v, kt=KT):
        return np.ascontiguousarray(np.asarray(v, f32).reshape(kt, 128).T)

    adaT = np.ascontiguousarray(
        np.asarray(inputs["ada_w"], f32).T.reshape(D, 9, D).transpose(1, 0, 2)
    ).astype(BF16_NP)

    shared = {
        "condT": np.ascontiguousarray(cond.T),
        "adaT": adaT,
        "adab": fm(inputs["ada_b"], 9 * KT),
        "qkvT": bfT(inputs["sa_qkv_w"]),
        "projT": bfT(inputs["sa_proj_w"]),
        "caqT": bfT(inputs["ca_q_w"]),
        "cakT": bfT(inputs["ca_k_w"]),
        "cavT": bfT(inputs["ca_v_w"]),
        "caprojT": bfT(inputs["ca_proj_w"]),
        "w1T": bfT(inputs["mlp_w1"]),
        "w3T": bfT(inputs["mlp_w3"]),
        "w2T": bfT(inputs["mlp_w2"]),
        "n1w": fm(inputs["norm1_w"]),
        "n2w": fm(inputs["norm2_w"]),
        "n3w": fm(inputs["norm3_w"]),
        "cnw": fm(inputs["cond_norm_w"]),
        "psab": fm(inputs["sa_proj_b"]),
        "pcab": fm(inputs["ca_proj_b"]),
        "cvec": fm(c),
        "caknw": np.ascontiguousarray(
            np.concatenate([np.asarray(inputs["ca_kn_w"], f32)] * 2)[:, None]),
    }
    bones64 = np.zeros((128, 128), BF16_NP)
    bones64[0:64, 0:64] = 1
    bones64[64:128, 64:128] = 1
    bones2 = np.zeros((2, 128), BF16_NP)
    bones2[0, 0:64] = 1
    bones2[1, 64:128] = 1
    shared["bones64"] = bones64
    shared["bones2"] = bones2
    shared["ones128"] = np.ones((128, 128), BF16_NP)
    shared["ident"] = np.eye(128, dtype=BF16_NP)

    sgn = np.concatenate([-np.ones(32), np.ones(32)]).astype(f32)

    def rope_tabs(w):
        w = np.asarray(w, f32)
        sww = np.concatenate([w[32:], w[:32]])
        cos_all, sin_all = [], []
        for cc in range(NCORES):
            sl = slice(cc * NT, (cc + 1) * NT)
            co = cos_t[sl].T * w[:, None]
            si = sin_t[sl].T * sgn[:, None] * sww[:, None]
            cos_all.append(np.ascontiguousarray(
                np.concatenate([co, co], 0), dtype=f32))
            sin_all.append(np.ascontiguousarray(
                np.concatenate([si, si], 0), dtype=f32))
        return cos_all, sin_all

    cosq_a, sinq_a = rope_tabs(inputs["sa_qn_w"])
    cosk_a, sink_a = rope_tabs(inputs["sa_kn_w"])
    cosqc_a, sinqc_a = rope_tabs(inputs["ca_qn_w"])

    in_maps = []
    for cc in range(NCORES):
        m = dict(shared)
        m["xT"] = np.ascontiguousarray(x[cc * NT:(cc + 1) * NT].T)
        m["cosq"], m["sinq"] = cosq_a[cc], sinq_a[cc]
        m["cosk"], m["sink"] = cosk_a[cc], sink_a[cc]
        m["cosqc"], m["sinqc"] = cosqc_a[cc], sinqc_a[cc]
        in_maps.append(m)
    return in_maps


_CACHED = {}


def kernel(**inputs):
    if "nc" not in _CACHED:
        _CACHED["nc"] = build_program()
    nc = _CACHED["nc"]
    in_maps = prep_inputs(inputs)
    res = run_bass_kernel_spmd(nc, in_maps, list(range(NCORES)))
    outs = [res.results[cc]["outT"] for cc in range(NCORES)]
    full = np.concatenate([o.T for o in outs], axis=0)[None]
    return np.ascontiguousarray(full.astype(np.float32))
